# revision 11
# baseline (speedup 1.0000x reference)
"""FlowNet correlation kernel for Trainium2 (Bass/Tile), 8-core data-parallel.

out[b, j*21+i, y, x] = (1/C) * sum_c x1[b,c,y,x] * pad20(x2)[b,c, y+2j, x+2i]

Strategy (per core = one batch element):
  - Inputs are cast to bf16 on the host, output is bf16 on device and
    upcast on the host: halves every DRAM transfer.
  - Parity-split y and x (displacement stride 2); block pairs (y,x) into
    128-partition stationary tiles (RY=8 parity-rows x RX=16 parity-cols),
    pair index p = dy*RX + dx (dy-major).
  - PE computes the banded Gram rectangle per block in bf16:
    psum[pair, (a,b)] = <x1[:,pair], x2[:, halo(a,b)]>, halo 28x36.
  - Escape PSUM->SBUF bf16 with the 1/C scale (split DVE/ACT).
  - Per-pair 21x21 window gather bounces through DRAM (flat DRAM strides
    are unrestricted; SBUF partition steps must be whole rows): 8 slab
    writes/block with a dx-shear put pair q's window at base S*q + 36j+i.
    The read back fetches whole 741-elem slabs (1482B runs, full DMA
    bandwidth); a strided on-chip copy compacts 36j+i -> 21j+i.
  - PE transpose (vs bf16 identity) flips E to [ji, pair] chunks.
  - Merge-copies interleave both parities into bf16 [ji, 16 rows, w] out
    tiles (one per gy covering 16 consecutive rows -> 5120B DMA runs).
  - Slab writes and the read back ride different HWDGE rings so the
    write->read dependency always gets a real semaphore.
"""

import numpy as np
import ml_dtypes

import concourse.bacc as bacc
import concourse.bass as bass
import concourse.mybir as mybir
import concourse.tile as tile
from concourse.bass_utils import run_bass_kernel_spmd
from concourse.masks import make_identity

F32 = mybir.dt.float32
BF16 = mybir.dt.bfloat16

C = 256
H = 96
W = 160
NB = 8
J = 21          # taps per axis
PAD = 20
RY = 8          # parity rows per block
RX = 16         # parity cols per block
JI = J * J      # 441
JIPAD = 448
CHW = 112       # fold chunk width (JIPAD // 4)
SLAB = J * (RX + PAD)   # 756 contiguous elems per pair slab
SPAN = 36 * (J - 1) + J  # 741: last used slab offset (36*20+20) + 1
# Slab for pair p=(dy*RX+dx) holds rect rows [dy, dy+21); within it,
# E[p, (j,i)] sits at j*36 + dx + i. Storing the slab at base S*p - dx
# makes the window of pair q live at S*q + 36*j + i.
S = 768         # scratch stride per pair (>= SLAB + 15)


def build_nc(h=H, w=W, n_cores=NB):
    hp, wp = h // 2, w // 2
    gys, gxs = hp // RY, wp // RX
    ah, bw = RY + PAD, RX + PAD       # halo extents (28, 36)
    rect = ah * bw                    # 1008
    hw = h * w

    nc = bacc.Bacc("TRN2", target_bir_lowering=False, debug=False,
                   num_devices=n_cores)
    x1d = nc.dram_tensor("input1", [C, h, w], BF16, kind="ExternalInput")
    x2d = nc.dram_tensor("input2", [C, h, w], BF16, kind="ExternalInput")
    # 448 channels so the one-DMA-per-gy store can use all 4x112 chunk
    # partitions; the host slices [:441].
    outd = nc.dram_tensor("out", [JIPAD, h, w], BF16, kind="ExternalOutput")

    hwdge = [nc.sync, nc.scalar]      # the two HWDGE rings

    with tile.TileContext(nc) as tc:
        with (
            tc.tile_pool(name="x2pool", bufs=1) as x2pool,
            tc.tile_pool(name="x1pool", bufs=2) as x1pool,
            tc.tile_pool(name="identpool", bufs=1) as identpool,
            tc.tile_pool(name="rectpool", bufs=4) as rectpool,
            tc.tile_pool(name="epool", bufs=4) as epool,
            tc.tile_pool(name="erawpool", bufs=3) as erawpool,
            tc.tile_pool(name="outpool", bufs=2) as outpool,
            tc.tile_pool(name="dramscr", bufs=8, space="DRAM") as dramscr,
            tc.tile_pool(name="rectps", bufs=3, space="PSUM") as rectps,
            tc.tile_pool(name="foldps", bufs=2, space="PSUM") as foldps,
        ):
            ident = identpool.tile([128, 128], BF16)
            make_identity(nc, ident[:])

            # engines for the rect edge memsets, round-robin
            ms_engines = [nc.gpsimd, nc.vector]
            ms_idx = [0]

            def edge_memset(dst):
                ms_engines[ms_idx[0] % 2].memset(dst, 0.0)
                ms_idx[0] += 1

            # x2 in row-halves, low rows of both K-chunks first, so gy=0
            # matmuls (rows <= 37) start before the full 7.9MB lands.
            x2sb = x2pool.tile([128, 2, h, w], BF16)
            for half in range(2):
                r0 = half * (h // 2)
                r1 = r0 + h // 2
                for k in range(2):
                    hwdge[1].dma_start(
                        out=x2sb[:, k, r0:r1],
                        in_=x2d[k * 128:(k + 1) * 128, r0:r1])

            def load_x1(gy):
                # x1 rows for this group: one SWDGE load per K-chunk, then
                # a rearrange into block-major stationary tiles (walrus:
                # matmul weights APs must have ONE free dim). SWDGE keeps
                # the input stream off the two HWDGE rings. One 3D-AP
                # copy per (k, py, px) — (gx, ry, rx) folded into the AP.
                yb = 2 * RY * gy
                x1t = x1pool.tile([128, 2, 2 * RY, w], BF16, tag="x1t",
                                  bufs=1)
                for k in range(2):
                    nc.gpsimd.dma_start(
                        out=x1t[:, k],
                        in_=x1d[k * 128:(k + 1) * 128, yb:yb + 2 * RY])
                x1s = x1pool.tile([128, 2, 2, 2, gxs, RY * RX], BF16,
                                  tag="x1s", name=f"x1s{gy}")
                for k in range(2):
                    for py in range(2):
                        for px in range(2):
                            src = x1t[:, k, py::2, px::2].rearrange(
                                "p a (g b) -> p g a b", g=gxs)
                            dst = x1s[:, k, py, px].rearrange(
                                "p g (a b) -> p g a b", a=RY)
                            if k == 0:
                                nc.vector.tensor_copy(out=dst, in_=src)
                            else:
                                nc.scalar.copy(out=dst, in_=src)
                return x1s

            blk = 0
            x1s = load_x1(0)
            for gy in range(gys):
                yb = 2 * RY * gy                 # first of 16 real rows
                x1s_next = load_x1(gy + 1) if gy + 1 < gys else None

                ot = outpool.tile([CHW, 4, 2 * RY, w], BF16, tag="ot",
                                  name=f"ot{gy}")

                for py in range(2):
                    y0 = py + yb                 # first real y row (parity)
                    for px in range(2):
                        eng = hwdge[blk % 2]
                        eng2 = hwdge[(blk + 1) % 2]
                        blk += 1
                        # one mega-rect for all gxs blocks of this x-parity;
                        # sub-rect gx at free offset gx*rect
                        mrow = gxs * rect
                        rs = rectpool.tile([128, gxs, ah, bw], BF16,
                                           tag="rs")
                        rsap = rs[:]
                        alo = max(0, -(-(PAD - y0) // 2))
                        ahi = min(ah, (h - 1 - y0 + PAD) // 2 + 1)

                        for gx in range(gxs):
                            x0 = px + 2 * RX * gx
                            # valid halo ranges (rows r = y0 + 2a - 20,
                            # cols u = x0 + 2b - 20)
                            blo = max(0, -(-(PAD - x0) // 2))
                            bhi = min(bw, (w - 1 - x0 + PAD) // 2 + 1)
                            nb_ = bhi - blo

                            # psum rect in two bank-aligned halves: half hh
                            # holds a in [14hh, 14hh+14) at [512hh, ...)
                            rp = rectps.tile([128, 2, 512], F32, tag="rp")

                            # zero-fill clipped halo strips in SBUF rect
                            if alo > 0:
                                edge_memset(rs[:, gx, :alo, :])
                            if ahi < ah:
                                edge_memset(rs[:, gx, ahi:, :])
                            if blo > 0:
                                edge_memset(rs[:, gx, alo:ahi, :blo])
                            if bhi < bw:
                                edge_memset(rs[:, gx, alo:ahi, bhi:])

                            # banded Gram matmuls, K=256 in two 128-chunks,
                            # one matmul per psum-bank half per K-chunk
                            rpap = rp[:]
                            hranges = []
                            for hh in range(2):
                                a0 = max(alo, 14 * hh)
                                a1 = min(ahi, 14 * (hh + 1))
                                if a0 >= a1:
                                    continue
                                hranges.append((hh, a0, a1))
                                na = a1 - a0
                                pout = bass.AP(
                                    tensor=rpap.tensor,
                                    offset=rpap.offset + 512 * hh,
                                    ap=[[1024, 128], [1, na * nb_]])
                                for k in range(2):
                                    lhsT = x1s[:, k, py, px, gx]
                                    r0 = y0 + 2 * a0 - PAD
                                    u0 = x0 + 2 * blo - PAD
                                    rhs = x2sb[:, k,
                                               r0:r0 + 2 * na - 1:2,
                                               u0:u0 + 2 * nb_ - 1:2]
                                    nc.tensor.matmul(
                                        pout, lhsT, rhs,
                                        start=(k == 0), stop=(k == 1))

                            # escape PSUM -> SBUF bf16 with 1/C scale
                            # (half 0 on DVE, half 1 on ACT)
                            for hh, a0, a1 in hranges:
                                na = a1 - a0
                                psrc = bass.AP(
                                    tensor=rpap.tensor,
                                    offset=rpap.offset + 512 * hh,
                                    ap=[[1024, 128], [nb_, na], [1, nb_]])
                                if hh == 0:
                                    nc.vector.tensor_scalar_mul(
                                        rs[:, gx, a0:a1, blo:bhi],
                                        psrc, 1.0 / C)
                                else:
                                    nc.scalar.mul(
                                        rs[:, gx, a0:a1, blo:bhi],
                                        psrc, 1.0 / C)

                        # gather via DRAM bounce, all gxs sub-rects at
                        # once. Leg 1 (8 DMAs): per dy-group g, partitions
                        # [16g, 16g+16) share slab rows [g, g+21); slab of
                        # scratch-pair q = gx*128 + p goes to base S*q - dx
                        # so its window sits at S*q + 36j + i.
                        scr = dramscr.tile([S * 128 * gxs], BF16, tag="scr")
                        scrap = scr[:]
                        for g in range(RY):
                            ssrc = bass.AP(
                                tensor=rsap.tensor,
                                offset=rsap.offset
                                + RX * g * mrow + g * bw,
                                ap=[[mrow, RX], [rect, gxs], [1, SLAB]])
                            sdst = bass.AP(
                                tensor=scrap.tensor,
                                offset=scrap.offset + RX * S * g,
                                ap=[[S - 1, RX], [128 * S, gxs], [1, SLAB]])
                            eng.dma_start(out=sdst, in_=ssrc)

                        # Leg 2: whole-slab read back for all gxs blocks
                        # in ONE DMA (1482B runs) on the other ring.
                        eraw = erawpool.tile([128, gxs, S], BF16,
                                             tag="eraw")
                        gsrc = bass.AP(
                            tensor=scrap.tensor,
                            offset=scrap.offset,
                            ap=[[S, 128], [128 * S, gxs], [1, SPAN]])
                        gdst = bass.AP(
                            tensor=eraw[:].tensor,
                            offset=eraw[:].offset,
                            ap=[[gxs * S, 128], [S, gxs], [1, SPAN]])
                        eng2.dma_start(out=gdst, in_=gsrc)
                        erap = eraw[:]

                        for gx in range(gxs):
                            x0 = px + 2 * RX * gx
                            # on-chip window compaction 36j+i -> 21j+i
                            et = epool.tile([128, JIPAD], BF16, tag="et")
                            nc.vector.memset(et[:, JI:], 0.0)
                            csrc = bass.AP(
                                tensor=erap.tensor,
                                offset=erap.offset + gx * S,
                                ap=[[gxs * S, 128], [36, J], [1, J]])
                            nc.vector.tensor_copy(
                                out=et[:, :JI].rearrange(
                                    "p (j i) -> p j i", j=J),
                                in_=csrc)

                            # PE transpose chunks (all 4 share one PSUM
                            # bank) + merge into the gy out tile
                            # (pairs dy-major: free dims (dy RX, dx 1));
                            # y = py + 2dy within the 16-row tile, x = px
                            # + 2(16gx + dx): merges split Pool/ACT/DVE/ACT
                            fp = foldps.tile([CHW, 4, 128], BF16, tag="fp")
                            for ci in range(4):
                                nj = min(CHW, JI - ci * CHW)
                                nc.tensor.transpose(
                                    fp[:, ci],
                                    et[:, ci * CHW:(ci + 1) * CHW],
                                    ident[:])
                                fpap = fp[:]
                                msrc = bass.AP(
                                    tensor=fpap.tensor,
                                    offset=fpap.offset + ci * 128,
                                    ap=[[4 * 128, nj], [RX, RY], [1, RX]])
                                otap = ot[:]
                                mdst = bass.AP(
                                    tensor=otap.tensor,
                                    offset=otap.offset
                                    + ci * 2 * RY * w + py * w + x0,
                                    ap=[[4 * 2 * RY * w, nj], [2 * w, RY],
                                        [2, RX]])
                                if ci % 2 == 0:
                                    nc.vector.tensor_copy(out=mdst,
                                                          in_=msrc)
                                else:
                                    nc.scalar.copy(out=mdst, in_=msrc)

                # DMA out via SWDGE, split in two so neither transfer
                # holds the DMA engines too long; 16 consecutive rows per
                # channel (5120B runs); channels [441,448) are junk the
                # host drops.
                for cih in range(4):
                    dst = bass.AP(
                        tensor=outd,
                        offset=cih * CHW * hw + yb * w,
                        ap=[[hw, CHW], [1, 2 * RY * w]])
                    nc.gpsimd.dma_start(out=dst, in_=ot[:, cih])
                x1s = x1s_next

    nc.compile()
    return nc


_NC_CACHE = {}


def _get_nc(h, w, n_cores):
    key = (h, w, n_cores)
    if key not in _NC_CACHE:
        _NC_CACHE[key] = build_nc(h, w, n_cores)
    return _NC_CACHE[key]


def kernel(input1, input2):
    input1 = np.asarray(input1)
    input2 = np.asarray(input2)
    b, c, h, w = input1.shape
    assert c == C
    nc = _get_nc(h, w, b)
    bf = ml_dtypes.bfloat16
    in_maps = [
        {"input1": np.ascontiguousarray(input1[i]).astype(bf),
         "input2": np.ascontiguousarray(input2[i]).astype(bf)}
        for i in range(b)
    ]
    res = run_bass_kernel_spmd(nc, in_maps, core_ids=list(range(b)))
    return np.stack([res.results[i]["out"][:JI].astype(np.float32)
                     for i in range(b)])



# revision 12
# speedup vs baseline: 1.0149x; 1.0149x over previous
"""FlowNet correlation kernel for Trainium2 (Bass/Tile), 8-core data-parallel.

out[b, j*21+i, y, x] = (1/C) * sum_c x1[b,c,y,x] * pad20(x2)[b,c, y+2j, x+2i]

Strategy (per core = one batch element):
  - Inputs are cast to bf16 on the host, output is bf16 on device and
    upcast on the host: halves every DRAM transfer.
  - Parity-split y and x (displacement stride 2); block pairs (y,x) into
    128-partition stationary tiles (RY=8 parity-rows x RX=16 parity-cols),
    pair index p = dy*RX + dx (dy-major).
  - PE computes the banded Gram rectangle per block in bf16:
    psum[pair, (a,b)] = <x1[:,pair], x2[:, halo(a,b)]>, halo 28x36.
  - Escape PSUM->SBUF bf16 with the 1/C scale (split DVE/ACT).
  - Per-pair 21x21 window gather bounces through DRAM (flat DRAM strides
    are unrestricted; SBUF partition steps must be whole rows): 8 slab
    writes/block with a dx-shear put pair q's window at base S*q + 36j+i.
    The read back fetches whole 741-elem slabs (1482B runs, full DMA
    bandwidth); a strided on-chip copy compacts 36j+i -> 21j+i.
  - PE transpose (vs bf16 identity) flips E to [ji, pair] chunks.
  - Merge-copies interleave both parities into bf16 [ji, 16 rows, w] out
    tiles (one per gy covering 16 consecutive rows -> 5120B DMA runs).
  - Slab writes and the read back ride different HWDGE rings so the
    write->read dependency always gets a real semaphore.
"""

import numpy as np
import ml_dtypes

import concourse.bacc as bacc
import concourse.bass as bass
import concourse.mybir as mybir
import concourse.tile as tile
from concourse.bass_utils import run_bass_kernel_spmd
from concourse.masks import make_identity

F32 = mybir.dt.float32
BF16 = mybir.dt.bfloat16

C = 256
H = 96
W = 160
NB = 8
J = 21          # taps per axis
PAD = 20
RY = 8          # parity rows per block
RX = 16         # parity cols per block
JI = J * J      # 441
JIPAD = 448
CHW = 112       # fold chunk width (JIPAD // 4)
SLAB = J * (RX + PAD)   # 756 contiguous elems per pair slab
SPAN = 36 * (J - 1) + J  # 741: last used slab offset (36*20+20) + 1
# Slab for pair p=(dy*RX+dx) holds rect rows [dy, dy+21); within it,
# E[p, (j,i)] sits at j*36 + dx + i. Storing the slab at base S*p - dx
# makes the window of pair q live at S*q + 36*j + i.
S = 768         # scratch stride per pair (>= SLAB + 15)


def build_nc(h=H, w=W, n_cores=NB):
    hp, wp = h // 2, w // 2
    gys, gxs = hp // RY, wp // RX
    ah, bw = RY + PAD, RX + PAD       # halo extents (28, 36)
    rect = ah * bw                    # 1008
    hw = h * w

    nc = bacc.Bacc("TRN2", target_bir_lowering=False, debug=False,
                   num_devices=n_cores)
    x1d = nc.dram_tensor("input1", [C, h, w], BF16, kind="ExternalInput")
    x2d = nc.dram_tensor("input2", [C, h, w], BF16, kind="ExternalInput")
    # 448 channels so the one-DMA-per-gy store can use all 4x112 chunk
    # partitions; the host slices [:441].
    outd = nc.dram_tensor("out", [JIPAD, h, w], BF16, kind="ExternalOutput")

    hwdge = [nc.sync, nc.scalar]      # the two HWDGE rings

    with tile.TileContext(nc) as tc:
        with (
            tc.tile_pool(name="x2pool", bufs=1) as x2pool,
            tc.tile_pool(name="x1pool", bufs=2) as x1pool,
            tc.tile_pool(name="identpool", bufs=1) as identpool,
            tc.tile_pool(name="rectpool", bufs=4) as rectpool,
            tc.tile_pool(name="epool", bufs=4) as epool,
            tc.tile_pool(name="erawpool", bufs=3) as erawpool,
            tc.tile_pool(name="outpool", bufs=2) as outpool,
            tc.tile_pool(name="dramscr", bufs=8, space="DRAM") as dramscr,
            tc.tile_pool(name="rectps", bufs=3, space="PSUM") as rectps,
            tc.tile_pool(name="foldps", bufs=2, space="PSUM") as foldps,
        ):
            ident = identpool.tile([128, 128], BF16)
            make_identity(nc, ident[:])

            # engines for the rect edge memsets, round-robin
            ms_engines = [nc.gpsimd, nc.vector]
            ms_idx = [0]

            def edge_memset(dst):
                ms_engines[ms_idx[0] % 2].memset(dst, 0.0)
                ms_idx[0] += 1

            # x2 in row-halves, low rows of both K-chunks first, so gy=0
            # matmuls (rows <= 37) start before the full 7.9MB lands.
            x2sb = x2pool.tile([128, 2, h, w], BF16)
            for half in range(2):
                r0 = half * (h // 2)
                r1 = r0 + h // 2
                for k in range(2):
                    hwdge[1].dma_start(
                        out=x2sb[:, k, r0:r1],
                        in_=x2d[k * 128:(k + 1) * 128, r0:r1])

            def load_x1(gy):
                # x1 rows for this group: one SWDGE load per K-chunk, then
                # a rearrange into block-major stationary tiles (walrus:
                # matmul weights APs must have ONE free dim). SWDGE keeps
                # the input stream off the two HWDGE rings. One 3D-AP
                # copy per (k, py, px) — (gx, ry, rx) folded into the AP.
                yb = 2 * RY * gy
                x1t = x1pool.tile([128, 2, 2 * RY, w], BF16, tag="x1t",
                                  bufs=1)
                for k in range(2):
                    nc.gpsimd.dma_start(
                        out=x1t[:, k],
                        in_=x1d[k * 128:(k + 1) * 128, yb:yb + 2 * RY])
                x1s = x1pool.tile([128, 2, 2, 2, gxs, RY * RX], BF16,
                                  tag="x1s", name=f"x1s{gy}")
                for k in range(2):
                    for py in range(2):
                        for px in range(2):
                            src = x1t[:, k, py::2, px::2].rearrange(
                                "p a (g b) -> p g a b", g=gxs)
                            dst = x1s[:, k, py, px].rearrange(
                                "p g (a b) -> p g a b", a=RY)
                            if k == 0:
                                nc.vector.tensor_copy(out=dst, in_=src)
                            else:
                                nc.scalar.copy(out=dst, in_=src)
                return x1s

            mrow = gxs * rect

            def front_half(blk, x1s, ot, gy, py, px):
                """Matmuls -> escape -> slab writes -> readback DMA.
                Returns the state the back half needs."""
                eng = hwdge[blk % 2]
                eng2 = hwdge[(blk + 1) % 2]
                yb = 2 * RY * gy
                y0 = py + yb                     # first real y row (parity)
                # one mega-rect for all gxs blocks of this x-parity;
                # sub-rect gx at free offset gx*rect
                rs = rectpool.tile([128, gxs, ah, bw], BF16, tag="rs")
                rsap = rs[:]
                alo = max(0, -(-(PAD - y0) // 2))
                ahi = min(ah, (h - 1 - y0 + PAD) // 2 + 1)

                for gx in range(gxs):
                    x0 = px + 2 * RX * gx
                    # valid halo ranges (rows r = y0 + 2a - 20,
                    # cols u = x0 + 2b - 20)
                    blo = max(0, -(-(PAD - x0) // 2))
                    bhi = min(bw, (w - 1 - x0 + PAD) // 2 + 1)
                    nb_ = bhi - blo

                    # psum rect in two bank-aligned halves: half hh
                    # holds a in [14hh, 14hh+14) at [512hh, ...)
                    rp = rectps.tile([128, 2, 512], F32, tag="rp")

                    # zero-fill clipped halo strips in SBUF rect
                    if alo > 0:
                        edge_memset(rs[:, gx, :alo, :])
                    if ahi < ah:
                        edge_memset(rs[:, gx, ahi:, :])
                    if blo > 0:
                        edge_memset(rs[:, gx, alo:ahi, :blo])
                    if bhi < bw:
                        edge_memset(rs[:, gx, alo:ahi, bhi:])

                    # banded Gram matmuls, K=256 in two 128-chunks,
                    # one matmul per psum-bank half per K-chunk
                    rpap = rp[:]
                    hranges = []
                    for hh in range(2):
                        a0 = max(alo, 14 * hh)
                        a1 = min(ahi, 14 * (hh + 1))
                        if a0 >= a1:
                            continue
                        hranges.append((hh, a0, a1))
                        na = a1 - a0
                        pout = bass.AP(
                            tensor=rpap.tensor,
                            offset=rpap.offset + 512 * hh,
                            ap=[[1024, 128], [1, na * nb_]])
                        for k in range(2):
                            lhsT = x1s[:, k, py, px, gx]
                            r0 = y0 + 2 * a0 - PAD
                            u0 = x0 + 2 * blo - PAD
                            rhs = x2sb[:, k,
                                       r0:r0 + 2 * na - 1:2,
                                       u0:u0 + 2 * nb_ - 1:2]
                            nc.tensor.matmul(
                                pout, lhsT, rhs,
                                start=(k == 0), stop=(k == 1))

                    # escape PSUM -> SBUF bf16 with 1/C scale
                    # (half 0 on DVE, half 1 on ACT)
                    for hh, a0, a1 in hranges:
                        na = a1 - a0
                        psrc = bass.AP(
                            tensor=rpap.tensor,
                            offset=rpap.offset + 512 * hh,
                            ap=[[1024, 128], [nb_, na], [1, nb_]])
                        if hh == 0:
                            nc.vector.tensor_scalar_mul(
                                rs[:, gx, a0:a1, blo:bhi],
                                psrc, 1.0 / C)
                        else:
                            nc.scalar.mul(
                                rs[:, gx, a0:a1, blo:bhi],
                                psrc, 1.0 / C)

                # gather via DRAM bounce, all gxs sub-rects at
                # once. Leg 1 (8 DMAs): per dy-group g, partitions
                # [16g, 16g+16) share slab rows [g, g+21); slab of
                # scratch-pair q = gx*128 + p goes to base S*q - dx
                # so its window sits at S*q + 36j + i.
                scr = dramscr.tile([S * 128 * gxs], BF16, tag="scr")
                scrap = scr[:]
                for g in range(RY):
                    ssrc = bass.AP(
                        tensor=rsap.tensor,
                        offset=rsap.offset + RX * g * mrow + g * bw,
                        ap=[[mrow, RX], [rect, gxs], [1, SLAB]])
                    sdst = bass.AP(
                        tensor=scrap.tensor,
                        offset=scrap.offset + RX * S * g,
                        ap=[[S - 1, RX], [128 * S, gxs], [1, SLAB]])
                    eng.dma_start(out=sdst, in_=ssrc)

                # Leg 2: whole-slab read back for all gxs blocks
                # in ONE DMA (1482B runs) on the other ring.
                eraw = erawpool.tile([128, gxs, S], BF16, tag="eraw")
                gsrc = bass.AP(
                    tensor=scrap.tensor,
                    offset=scrap.offset,
                    ap=[[S, 128], [128 * S, gxs], [1, SPAN]])
                gdst = bass.AP(
                    tensor=eraw[:].tensor,
                    offset=eraw[:].offset,
                    ap=[[gxs * S, 128], [S, gxs], [1, SPAN]])
                eng2.dma_start(out=gdst, in_=gsrc)
                return eraw

            def back_half(eraw, ot, gy, py, px, last_of_gy):
                erap = eraw[:]
                yb = 2 * RY * gy
                for gx in range(gxs):
                    x0 = px + 2 * RX * gx
                    # on-chip window compaction 36j+i -> 21j+i
                    et = epool.tile([128, JIPAD], BF16, tag="et")
                    nc.vector.memset(et[:, JI:], 0.0)
                    csrc = bass.AP(
                        tensor=erap.tensor,
                        offset=erap.offset + gx * S,
                        ap=[[gxs * S, 128], [36, J], [1, J]])
                    nc.vector.tensor_copy(
                        out=et[:, :JI].rearrange(
                            "p (j i) -> p j i", j=J),
                        in_=csrc)

                    # PE transpose chunks (all 4 share one PSUM
                    # bank) + merge into the gy out tile
                    # (pairs dy-major: free dims (dy RX, dx 1));
                    # y = py + 2dy within the 16-row tile, x = px
                    # + 2(16gx + dx): merges split DVE/ACT
                    fp = foldps.tile([CHW, 4, 128], BF16, tag="fp")
                    for ci in range(4):
                        nj = min(CHW, JI - ci * CHW)
                        nc.tensor.transpose(
                            fp[:, ci],
                            et[:, ci * CHW:(ci + 1) * CHW],
                            ident[:])
                        fpap = fp[:]
                        msrc = bass.AP(
                            tensor=fpap.tensor,
                            offset=fpap.offset + ci * 128,
                            ap=[[4 * 128, nj], [RX, RY], [1, RX]])
                        otap = ot[:]
                        mdst = bass.AP(
                            tensor=otap.tensor,
                            offset=otap.offset
                            + ci * 2 * RY * w + py * w + x0,
                            ap=[[4 * 2 * RY * w, nj], [2 * w, RY],
                                [2, RX]])
                        if ci % 2 == 0:
                            nc.vector.tensor_copy(out=mdst, in_=msrc)
                        else:
                            nc.scalar.copy(out=mdst, in_=msrc)

                if last_of_gy:
                    # DMA out via SWDGE, one chunk per DMA so the bursts
                    # interleave with bounce traffic; 16 consecutive rows
                    # per channel (5120B runs); channels [441,448) are
                    # junk the host drops.
                    for cih in range(4):
                        dst = bass.AP(
                            tensor=outd,
                            offset=cih * CHW * hw + yb * w,
                            ap=[[hw, CHW], [1, 2 * RY * w]])
                        nc.gpsimd.dma_start(out=dst, in_=ot[:, cih])

            # software pipeline: quad n+1's front half (matmuls ->
            # bounce) is issued BEFORE quad n's back half (compact ->
            # transpose -> merge) so escapes never queue behind merges
            # on the in-order V/S engines while the bounce drains.
            quads = [(gy, py, px)
                     for gy in range(gys) for py in range(2)
                     for px in range(2)]
            x1s_cur = load_x1(0)
            x1s = {0: x1s_cur}
            ots = {}
            pending = None
            for blk, (gy, py, px) in enumerate(quads):
                if py == 0 and px == 0:
                    ots[gy] = outpool.tile([CHW, 4, 2 * RY, w], BF16,
                                           tag="ot", name=f"ot{gy}")
                    if gy + 1 < gys:
                        x1s[gy + 1] = load_x1(gy + 1)
                eraw = front_half(blk, x1s[gy], ots[gy], gy, py, px)
                if pending is not None:
                    back_half(*pending)
                pending = (eraw, ots[gy], gy, py, px,
                           py == 1 and px == 1)
                if py == 1 and px == 1 and gy - 1 in x1s:
                    del x1s[gy - 1]
            back_half(*pending)

    nc.compile()
    return nc


_NC_CACHE = {}


def _get_nc(h, w, n_cores):
    key = (h, w, n_cores)
    if key not in _NC_CACHE:
        _NC_CACHE[key] = build_nc(h, w, n_cores)
    return _NC_CACHE[key]


def kernel(input1, input2):
    input1 = np.asarray(input1)
    input2 = np.asarray(input2)
    b, c, h, w = input1.shape
    assert c == C
    nc = _get_nc(h, w, b)
    bf = ml_dtypes.bfloat16
    in_maps = [
        {"input1": np.ascontiguousarray(input1[i]).astype(bf),
         "input2": np.ascontiguousarray(input2[i]).astype(bf)}
        for i in range(b)
    ]
    res = run_bass_kernel_spmd(nc, in_maps, core_ids=list(range(b)))
    return np.stack([res.results[i]["out"][:JI].astype(np.float32)
                     for i in range(b)])

